# revision 2
# baseline (speedup 1.0000x reference)
"""Bidirectional Mamba block (nn_BiMamba) on 8 Trainium2 NeuronCores — v2.

Sharding: core c in 0..7 -> (batch b = c>>2, direction dir = (c>>1)&1,
channel-half dh = c&1).  Each core runs the full per-direction mamba
pipeline for its batch on 64 of the 128 d_inner channels, with the FULL
d_state=256 per channel (no state-half AllReduce).  Host permutes the
channel axis so each core's 64 channels are rows 0..63 of every tensor.

Scan layout: channels are packed 4 per tile — a scan tile [128, 512] holds
(4 channels x 32 states) in partitions and a 512-step time half in the free
dim; time halves are chained through the scan's per-partition initial value.
Per 4-channel group: one PE matmul broadcasts 4 dt rows (via a [4,128]
block stationary), exp runs on ACT straight from PSUM with a per-partition
|A| scale, b = dtx*B and m = h*C are bf16 DVE muls (2x mode), the scan runs
on Pool (GPSIMD) with a slice on DVE for balance, and y = sum over states
falls out of a PSUM-accumulated matmul with a sliding one-hot stationary.

The whole per-iteration body (fcc, both mamba blocks, LayerNorms,
collectives, output store) sits inside the nrep loop so the repeat-slope
timing in test.py measures the full kernel.
"""

import json
import math

import numpy as np
import ml_dtypes

import concourse.bass as bass
import concourse.mybir as mybir
import concourse.tile as tile

AF = mybir.ActivationFunctionType
ALU = mybir.AluOpType
F32 = mybir.dt.float32
BF16 = mybir.dt.bfloat16

# problem dims (hardcoded per task contract)
B, NSITE, NCELL, DIM = 2, 128, 8, 32
DM, DI, DS, DR, DCONV = 2 * DIM, 4 * DIM, 256, 4, 4
L = NSITE * NCELL            # 1024
DH = DI // 2                 # 64 channels per core
NG = DH // 4                 # 16 channel groups of 4
NQ = DS // 32                # 8 state-eighths of 32
N_CORES = 8
EPS = 1e-5
H = 512                      # time half

# scans with index % SCAN_DVE_MOD == SCAN_DVE_MOD-1 run on DVE, rest on Pool
SCAN_DVE_MOD = 6

# ---------------------------------------------------------------------------
# BIR post-processing: this walrus build accepts at most ONE sync wait per
# instruction; hoist excess waits onto standalone NoOp carriers.
# ---------------------------------------------------------------------------

def _split_waits(bir_json: bytes, maxw: int = 1) -> bytes:
    data = json.loads(bir_json)
    n = [0]

    def carrier(ins, waits):
        n[0] += 1
        return {
            "debug": ins.get("debug", 0),
            "engine": ins["engine"],
            "ins": [],
            "name": f"I-wsplit-{n[0]}",
            "opcode": "NoOp",
            "outs": [],
            "sync_info": {"on_update": [], "on_wait": waits},
        }

    for fn in data["functions"]:
        for blk in fn["blocks"]:
            out = []
            for ins in blk["instructions"]:
                si = ins.get("sync_info")
                if si and si.get("on_wait") and len(si["on_wait"]) > maxw:
                    waits = si["on_wait"]
                    extra, keep = waits[:-maxw], waits[-maxw:]
                    for i in range(0, len(extra), maxw):
                        out.append(carrier(ins, extra[i:i + maxw]))
                    si["on_wait"] = keep
                out.append(ins)
            blk["instructions"] = out
    return json.dumps(data).encode()


_orig_to_json_bytes = bass.Bass.to_json_bytes


def _patched_to_json_bytes(self, *a, **k):
    return _split_waits(_orig_to_json_bytes(self, *a, **k))


bass.Bass.to_json_bytes = _patched_to_json_bytes

# ---------------------------------------------------------------------------
# device program
# ---------------------------------------------------------------------------

def build_bass(nrep: int = 1, no_cc: bool = False, dbg: bool = False):
    nc = bass.Bass(num_devices=N_CORES)
    f32 = F32

    def din(name, shape, dtype=f32):
        return nc.dram_tensor(name, shape, dtype, kind="ExternalInput")

    emb_T = din("emb_T", [3 * DIM, L])
    pos_T = din("pos_T", [3 * DIM, L])
    fccT = din("fccT", [3 * DIM, DM], BF16)
    fccb = din("fccb", [DM, 1])
    inpxT = din("inpxT", [DM, DI], BF16)
    inpzT = din("inpzT", [DM, DH], BF16)
    convW = din("convW", [DI, DCONV])
    convB = din("convB", [DI, 1])
    xprojD = din("xprojD", [DI, DR], BF16)
    xprojB = din("xprojB", [DI, NQ * 128], BF16)   # negated, 4x-replicated rows
    xprojC = din("xprojC", [DI, NQ * 128], BF16)
    dtwT = din("dtwT", [DR, DH], BF16)
    ndtb = din("ndtb", [DH, 1])                    # minus dt_proj_b
    aposT = din("aposT", [128, NG * NQ])           # |A| per (partition, g*8+q)
    bc4in = din("bc4", [4, 128], BF16)             # bc4[k,p] = (p//32 == k)
    dcol = din("dcol", [DH, 1])
    lng = din("lng", [DM, 1])
    lnb = din("lnb", [DM, 1])
    outprojT = din("outprojT", [DH, DM], BF16)     # pre-scaled by 0.5
    idx_rev = din("idx_rev", [128, L // 16], mybir.dt.uint16)
    idx2 = din("idx2", [128, L // 16], mybir.dt.uint16)

    out_h = nc.dram_tensor("out", [DM, L], f32, kind="ExternalOutput")
    dbg_t = {}
    if dbg:
        for nm, shape, dt in [
            ("dbg_xc", [DI, L], f32), ("dbg_dthi", [DH, L], BF16),
            ("dbg_dtxn", [DH, L], BF16), ("dbg_brep0", [128, L], BF16),
            ("dbg_crep0", [128, L], BF16), ("dbg_a0", [128, L], BF16),
            ("dbg_b0", [128, L], BF16), ("dbg_h0q", [128, L], BF16),
            ("dbg_m0", [128, L], BF16), ("dbg_ys", [DM, L], f32),
            ("dbg_yo", [DM, L], f32), ("dbg_ysum", [DM, L], f32),
            ("dbg_hn1", [DM, L], f32), ("dbg_pdt", [128, L], f32),
        ]:
            dbg_t[nm] = nc.dram_tensor(nm, shape, dt, kind="ExternalOutput")

    with tile.TileContext(nc) as tc:
        with (
            tc.tile_pool(name="cst", bufs=1) as cst,
            tc.tile_pool(name="rep", bufs=1) as rpool,
            tc.tile_pool(name="blk", bufs=1) as blk,
            tc.tile_pool(name="hh", bufs=1) as hh,
            tc.tile_pool(name="pp", bufs=1) as pp,
            tc.tile_pool(name="rw", bufs=1) as rw,
            tc.tile_pool(name="stg", bufs=2) as stg,
            tc.tile_pool(name="lp", bufs=4) as lp,
            tc.tile_pool(name="hp", bufs=2) as hp,
            tc.tile_pool(name="pbc", bufs=4, space="PSUM") as pbc,
            tc.tile_pool(name="pys", bufs=1, space="PSUM") as pys,
            tc.tile_pool(name="pmm", bufs=2, space="PSUM") as pmm,
            tc.tile_pool(name="dram", bufs=2, space="DRAM") as dram,
        ):
            # ---- load constants / weights ----
            def load(t, shape, dtype=f32):
                s = cst.tile(shape, dtype, tag=t.name)
                nc.sync.dma_start(s[:], t[:])
                return s

            s_emb = load(emb_T, [3 * DIM, L])
            s_pos = load(pos_T, [3 * DIM, L])
            s_fccT = load(fccT, [3 * DIM, DM], BF16)
            s_fccb = load(fccb, [DM, 1])
            s_inpxT = load(inpxT, [DM, DI], BF16)
            s_inpzT = load(inpzT, [DM, DH], BF16)
            s_convW = load(convW, [DI, DCONV])
            s_convB = load(convB, [DI, 1])
            s_xprD = load(xprojD, [DI, DR], BF16)
            s_xprB = load(xprojB, [DI, NQ * 128], BF16)
            s_xprC = load(xprojC, [DI, NQ * 128], BF16)
            s_dtwT = load(dtwT, [DR, DH], BF16)
            s_ndtb = load(ndtb, [DH, 1])
            s_aposT = load(aposT, [128, NG * NQ])
            s_dcol = load(dcol, [DH, 1])
            s_lng = load(lng, [DM, 1])
            s_lnb = load(lnb, [DM, 1])
            s_outT = load(outprojT, [DH, DM], BF16)
            s_irev = load(idx_rev, [128, L // 16], mybir.dt.uint16)
            s_idx2 = load(idx2, [128, L // 16], mybir.dt.uint16)

            bc4 = load(bc4in, [4, 128], BF16)
            # zo2[p, 64 + p//32] = 1: sliding one-hot for the state reduce
            zo2 = cst.tile([128, 128], BF16, tag="zo2")
            nc.vector.memset(zo2[:], 0.0)
            for j in range(4):
                nc.vector.memset(zo2[32 * j:32 * j + 32, 64 + j:65 + j], 1.0)
            ones64c = cst.tile([DM, 1], BF16, tag="ones64c")
            nc.vector.memset(ones64c[:], 1.0)
            ones1x64 = cst.tile([1, DM], BF16, tag="ones1x64")
            nc.vector.memset(ones1x64[:], 1.0)
            eps_t = cst.tile([1, 1], f32, tag="eps_t")
            nc.vector.memset(eps_t[:], EPS)

            def halves():
                return [(v, v * H, (v + 1) * H) for v in range(2)]

            scan_ctr = [0]
            blk_ctr = [0]

            def mamba_block(u, base, hn, yo):
                """u: [128, L] f32 local-order input (rows 0:64 valid).
                base: [128, L] f32 canonical-order residual base.
                Writes hn [128, L] f32 canonical order (rows 0:64)."""
                # ---- in_proj (xc over all DI channels, z over ours) ----
                u16 = blk.tile([DM, L], BF16, tag="u16")
                nc.vector.tensor_copy(u16[:], u[0:DM, :])
                xcpad = blk.tile([DI, DCONV - 1 + L], f32, tag="xcpad")
                nc.vector.memset(xcpad[:, 0:DCONV - 1], 0.0)
                for _, a, b2 in halves():
                    p = pmm.tile([DI, H], f32, tag="pmm")
                    nc.tensor.matmul(p[:], s_inpxT[:], u16[:, a:b2],
                                     start=True, stop=True)
                    nc.scalar.activation(xcpad[:, DCONV - 1 + a:DCONV - 1 + b2],
                                         p[:], AF.Copy)
                zg = blk.tile([DH, L], f32, tag="zg")
                for _, a, b2 in halves():
                    p = pmm.tile([DI, H], f32, tag="pmm")
                    nc.tensor.matmul(p[0:DH, :], s_inpzT[:], u16[:, a:b2],
                                     start=True, stop=True)
                    sigz = pp.tile([DH, H], f32, tag="sigz")
                    nc.scalar.activation(sigz[:], p[0:DH, :], AF.Sigmoid)
                    nc.vector.tensor_mul(zg[:, a:b2], sigz[:], p[0:DH, :])

                # ---- depthwise causal conv + silu ----
                cv = blk.tile([DI, L], f32, tag="cv")
                cv2 = blk.tile([DI, L], f32, tag="cv2")
                nc.vector.tensor_scalar(cv[:], xcpad[:, 0:L], s_convW[:, 0:1],
                                        s_convB[:, 0:1], ALU.mult, ALU.add)
                nc.vector.scalar_tensor_tensor(cv2[:], xcpad[:, 1:1 + L],
                                               s_convW[:, 1:2], cv[:],
                                               ALU.mult, ALU.add)
                nc.vector.scalar_tensor_tensor(cv[:], xcpad[:, 2:2 + L],
                                               s_convW[:, 2:3], cv2[:],
                                               ALU.mult, ALU.add)
                nc.vector.scalar_tensor_tensor(cv2[:], xcpad[:, 3:3 + L],
                                               s_convW[:, 3:4], cv[:],
                                               ALU.mult, ALU.add)
                sigc = blk.tile([DI, L], f32, tag="cv")   # reuse cv buffer
                nc.scalar.activation(sigc[:], cv2[:], AF.Sigmoid)
                xc = blk.tile([DI, L], f32, tag="xc")
                nc.vector.tensor_mul(xc[:], cv2[:], sigc[:])
                xc16 = blk.tile([DI, L], BF16, tag="xc16")
                nc.vector.tensor_copy(xc16[:], xc[:])

                # ---- x_proj dt-rank rows ----
                dtr16 = blk.tile([DR, L], BF16, tag="dtr16")
                for _, a, b2 in halves():
                    p = pmm.tile([DI, H], f32, tag="pmm")
                    nc.tensor.matmul(p[0:DR, :], s_xprD[:], xc16[:, a:b2],
                                     start=True, stop=True)
                    nc.scalar.activation(dtr16[:, a:b2], p[0:DR, :], AF.Copy)

                # ---- dt (log-sigmoid form): dtn = -softplus(lin+b) ----
                sdt = blk.tile([DH, L], f32, tag="sdt")
                for _, a, b2 in halves():
                    p = pmm.tile([DI, H], f32, tag="pmm")
                    nc.tensor.matmul(p[0:DH, :], s_dtwT[:], dtr16[:, a:b2],
                                     start=True, stop=True)
                    nc.scalar.activation(sdt[:, a:b2], p[0:DH, :], AF.Sigmoid,
                                         scale=-1.0, bias=s_ndtb[:, 0:1])
                dthi = blk.tile([DH, L], BF16, tag="dthi")
                nc.scalar.activation(dthi[:], sdt[:], AF.Ln)
                dtxn = blk.tile([DH, L], BF16, tag="dtxn")
                nc.vector.tensor_mul(dtxn[:], dthi[:], xc16[0:DH, :])

                # ---- B (negated) and C, replicated 4x across partitions ----
                breps, creps = [], []
                evac_rr = [0]

                def evac(dst, src):
                    k = evac_rr[0] % 3
                    evac_rr[0] += 1
                    if k == 0:
                        nc.vector.tensor_copy(dst, src)
                    elif k == 1:
                        nc.scalar.activation(dst, src, AF.Copy)
                    else:
                        nc.gpsimd.tensor_copy(dst, src)

                for q in range(NQ):
                    bt = rpool.tile([128, L], BF16, tag=f"brep{q}")
                    ct = rpool.tile([128, L], BF16, tag=f"crep{q}")
                    for _, a, b2 in halves():
                        p = pmm.tile([DI, H], f32, tag="pmm")
                        nc.tensor.matmul(p[:], s_xprB[:, 128 * q:128 * (q + 1)],
                                         xc16[:, a:b2], start=True, stop=True)
                        evac(bt[:, a:b2], p[:])
                        p2 = pmm.tile([DI, H], f32, tag="pmm")
                        nc.tensor.matmul(p2[:], s_xprC[:, 128 * q:128 * (q + 1)],
                                         xc16[:, a:b2], start=True, stop=True)
                        evac(ct[:, a:b2], p2[:])
                    breps.append(bt)
                    creps.append(ct)

                first_blk = blk_ctr[0] == 0
                blk_ctr[0] += 1
                if dbg and first_blk:
                    nc.sync.dma_start(dbg_t["dbg_xc"][:], xc[:])
                    nc.sync.dma_start(dbg_t["dbg_dthi"][:], dthi[:])
                    nc.sync.dma_start(dbg_t["dbg_dtxn"][:], dtxn[:])
                    nc.sync.dma_start(dbg_t["dbg_brep0"][:], breps[0][:])
                    nc.sync.dma_start(dbg_t["dbg_crep0"][:], creps[0][:])
                # ---- selective scan: 16 groups x 2 halves x 8 eighths ----
                ys = [pys.tile([DM, H], f32, tag=f"ys{v}", name=f"ys{v}")
                      for v in range(2)]
                for g in range(NG):
                    sthi = stg.tile([4, L], BF16, tag="sthi")
                    nc.sync.dma_start(sthi[:], dthi[4 * g:4 * g + 4, :])
                    stdx = stg.tile([4, L], BF16, tag="stdx")
                    nc.sync.dma_start(stdx[:], dtxn[4 * g:4 * g + 4, :])
                    hprev = [None] * NQ
                    for v, a, b2 in halves():
                        pdt = pbc.tile([128, H], f32, tag="pbc")
                        nc.tensor.matmul(pdt[:], bc4[:], sthi[0:4, a:b2],
                                         start=True, stop=True)
                        pdx = pbc.tile([128, H], f32, tag="pbc")
                        nc.tensor.matmul(pdx[:], bc4[:], stdx[0:4, a:b2],
                                         start=True, stop=True)
                        dxb = lp.tile([128, H], BF16, tag="dxb")
                        nc.scalar.activation(dxb[:], pdx[:], AF.Copy)
                        for q in range(NQ):
                            col = g * NQ + q
                            a_t = lp.tile([128, H], BF16, tag="a")
                            nc.scalar.activation(a_t[:], pdt[:], AF.Exp,
                                                 scale=s_aposT[:, col:col + 1])
                            b_t = lp.tile([128, H], BF16, tag="b")
                            nc.vector.tensor_mul(b_t[:], dxb[:],
                                                 breps[q][:, a:b2])
                            h_t = hp.tile([128, H], BF16, tag=f"h{q}")
                            i = scan_ctr[0]
                            scan_ctr[0] += 1
                            eng = (nc.vector
                                   if i % SCAN_DVE_MOD == SCAN_DVE_MOD - 1
                                   else nc.gpsimd)
                            init = 0.0 if v == 0 else hprev[q][:, H - 1:H]
                            eng.tensor_tensor_scan(h_t[:], a_t[:], b_t[:],
                                                   init, ALU.mult, ALU.add)
                            hprev[q] = h_t
                            m_t = lp.tile([128, H], BF16, tag="m")
                            nc.vector.tensor_mul(m_t[:], h_t[:],
                                                 creps[q][:, a:b2])
                            if dbg and first_blk and g == 0 and q == 0:
                                nc.sync.dma_start(dbg_t["dbg_a0"][:, a:b2], a_t[:])
                                nc.sync.dma_start(dbg_t["dbg_b0"][:, a:b2], b_t[:])
                                nc.sync.dma_start(dbg_t["dbg_h0q"][:, a:b2], h_t[:])
                                nc.sync.dma_start(dbg_t["dbg_m0"][:, a:b2], m_t[:])
                            nc.tensor.matmul(
                                ys[v][:], zo2[:, 64 - 4 * g:128 - 4 * g],
                                m_t[:], start=(g == 0 and q == 0),
                                stop=(g == NG - 1 and q == NQ - 1),
                                skip_group_check=True)

                if dbg and first_blk:
                    for v, a, b2 in halves():
                        ysc = pp.tile([DM, H], f32, tag="ysc", name="ysc")
                        nc.scalar.activation(ysc[:], ys[v][:], AF.Copy)
                        nc.sync.dma_start(dbg_t["dbg_ys"][:, a:b2], ysc[:])
                # ---- gate + out_proj ----
                gated16 = pp.tile([DH, L], BF16, tag="gated16")
                for v, a, b2 in halves():
                    g1 = pp.tile([DH, H], f32, tag="g1")
                    nc.vector.scalar_tensor_tensor(g1[:], xc[0:DH, a:b2],
                                                   s_dcol[:, 0:1], ys[v][:],
                                                   ALU.mult, ALU.add)
                    nc.vector.tensor_mul(gated16[:, a:b2], g1[:], zg[:, a:b2])
                for _, a, b2 in halves():
                    p = pmm.tile([DI, H], f32, tag="pmm")
                    nc.tensor.matmul(p[0:DM, :], s_outT[:], gated16[:, a:b2],
                                     start=True, stop=True)
                    nc.scalar.activation(yo[0:DM, a:b2], p[0:DM, :], AF.Copy)
                yc = pp.tile([128, L], f32, tag="yc")
                nc.gpsimd.indirect_copy(yc[:], yo[:], s_irev[:], True)
                if dbg and first_blk:
                    nc.sync.dma_start(dbg_t["dbg_yo"][:], yo[0:DM, :])

                # ---- AllReduce over the 4-core batch group: 2*(yf+yb),
                #      out_proj pre-scaled 0.25 -> (yf+yb)/2, plus the
                #      channel-half partial sums of out_proj ----
                cc2_in = dram.tile([DM, L], f32, tag="cc2i")
                cc2_out = dram.tile([DM, L], f32, tag="cc2o")
                nc.sync.dma_start(cc2_in[:], yc[0:DM, :])
                if no_cc:
                    nc.sync.dma_start(cc2_out[:], cc2_in[:])
                else:
                    nc.gpsimd.collective_compute(
                        "AllReduce", ALU.add,
                        replica_groups=[[0, 1, 2, 3], [4, 5, 6, 7]],
                        ins=[cc2_in.opt()], outs=[cc2_out.opt()])
                ysum = pp.tile([DM, L], f32, tag="ysum")
                nc.sync.dma_start(ysum[:], cc2_out[:])

                # ---- residual + layernorm (canonical order) ----
                if dbg and first_blk:
                    nc.sync.dma_start(dbg_t["dbg_ysum"][:], ysum[:])
                rsd = pp.tile([DM, L], f32, tag="rsd")
                nc.vector.tensor_add(rsd[:], base[0:DM, :], ysum[:])
                rsd16 = pp.tile([DM, L], BF16, tag="rsd16")
                nc.vector.tensor_copy(rsd16[:], rsd[:])
                sq16 = pp.tile([DM, L], BF16, tag="sq16")
                nc.vector.tensor_mul(sq16[:], rsd16[:], rsd16[:])
                mu16 = rw.tile([1, L], BF16, tag="mu16")
                ex2 = rw.tile([1, L], f32, tag="ex2")
                for _, a, b2 in halves():
                    p = pmm.tile([DI, H], f32, tag="pmm")
                    nc.tensor.matmul(p[0:1, :], ones64c[:], rsd16[:, a:b2],
                                     start=True, stop=True)
                    nc.scalar.activation(mu16[0:1, a:b2], p[0:1, :], AF.Copy,
                                         scale=1.0 / DM)
                    p2 = pmm.tile([DI, H], f32, tag="pmm")
                    nc.tensor.matmul(p2[0:1, :], ones64c[:], sq16[:, a:b2],
                                     start=True, stop=True)
                    nc.scalar.activation(ex2[0:1, a:b2], p2[0:1, :], AF.Copy,
                                         scale=1.0 / DM)
                musq = rw.tile([1, L], f32, tag="musq")
                nc.vector.tensor_mul(musq[:], mu16[:], mu16[:])
                varv = rw.tile([1, L], f32, tag="varv")
                nc.vector.tensor_sub(varv[:], ex2[:], musq[:])
                lvar = rw.tile([1, L], f32, tag="musq", name="lvar")
                nc.scalar.activation(lvar[:], varv[:], AF.Ln,
                                     bias=eps_t[0:1, 0:1])
                rstd16 = rw.tile([1, L], BF16, tag="rstd16")
                nc.scalar.activation(rstd16[:], lvar[:], AF.Exp, scale=-0.5)
                for _, a, b2 in halves():
                    p = pmm.tile([DI, H], f32, tag="pmm")
                    nc.tensor.matmul(p[0:DM, :], ones1x64[:], mu16[0:1, a:b2],
                                     start=True, stop=True)
                    cen = pp.tile([DM, H], f32, tag="cen")
                    nc.vector.tensor_sub(cen[:], rsd[:, a:b2], p[0:DM, :])
                    p2 = pmm.tile([DI, H], f32, tag="pmm")
                    nc.tensor.matmul(p2[0:DM, :], ones1x64[:],
                                     rstd16[0:1, a:b2], start=True, stop=True)
                    nrm = pp.tile([DM, H], f32, tag="nrm")
                    nc.vector.tensor_mul(nrm[:], cen[:], p2[0:DM, :])
                    nc.vector.tensor_scalar(hn[0:DM, a:b2], nrm[:],
                                            s_lng[:, 0:1], s_lnb[:, 0:1],
                                            ALU.mult, ALU.add)
                return hn

            # tiles whose rows 64:128 are read (indirect_copy) but never
            # written by compute: allocate once, zero the garbage rows.
            h0 = hh.tile([128, L], f32, tag="h0")
            hn1 = hh.tile([128, L], f32, tag="hn1")
            hn2 = hh.tile([128, L], f32, tag="hn2")
            yo = pp.tile([128, L], f32, tag="yo")
            for tl in (h0, hn1, hn2, yo):
                nc.vector.memset(tl[DM:128, :], 0.0)

            for rep in range(nrep):
                # ---- embedding + fcc + relu ----
                ep = pp.tile([3 * DIM, L], f32, tag="ep")
                nc.vector.tensor_add(ep[:], s_emb[:], s_pos[:])
                ep16 = pp.tile([3 * DIM, L], BF16, tag="ep16")
                nc.vector.tensor_copy(ep16[:], ep[:])
                for _, a, b2 in halves():
                    p = pmm.tile([DI, H], f32, tag="pmm")
                    nc.tensor.matmul(p[0:DM, :], s_fccT[:], ep16[:, a:b2],
                                     start=True, stop=True)
                    nc.vector.tensor_scalar(h0[0:DM, a:b2], p[0:DM, :],
                                            s_fccb[:, 0:1], 0.0, ALU.add,
                                            ALU.max)
                h0cw = hh.tile([128, L], f32, tag="h0cw")
                nc.gpsimd.indirect_copy(h0cw[:], h0[:], s_irev[:], True)

                h2 = mamba_block(h0, h0cw, hn1, yo)
                if dbg:
                    nc.sync.dma_start(dbg_t["dbg_hn1"][:], h2[0:DM, :])

                # ---- transition: site-major -> cell-major ----
                h2t2 = hh.tile([DM, L], f32, tag="h2t2")
                nc.vector.tensor_copy(
                    h2t2[:].rearrange("p (c s) -> p c s", s=NSITE),
                    h2[0:DM, :].rearrange("p (s c) -> p s c",
                                          c=NCELL).transpose([0, 2, 1]))
                u2 = hh.tile([128, L], f32, tag="u2")
                nc.gpsimd.indirect_copy(u2[:], h2[:], s_idx2[:], True)

                h3 = mamba_block(u2, h2t2, hn2, yo)
                nc.sync.dma_start(out_h[:], h3[0:DM, :])

    return nc


# ---------------------------------------------------------------------------
# host side
# ---------------------------------------------------------------------------

def _pos_enc(D, Hh, W):
    pe = np.zeros((D, Hh, W), np.float32)
    dm = D // 2
    div = np.exp(np.arange(0, dm, 2, dtype=np.float32) * -(math.log(10000.0) / dm))
    pw = np.arange(W, dtype=np.float32)[:, None]
    ph = np.arange(Hh, dtype=np.float32)[:, None]
    pe[0:dm:2] = np.broadcast_to(np.sin(pw * div).T[:, None, :], (dm // 2, Hh, W))
    pe[1:dm:2] = np.broadcast_to(np.cos(pw * div).T[:, None, :], (dm // 2, Hh, W))
    pe[dm::2] = np.broadcast_to(np.sin(ph * div).T[:, :, None], (dm // 2, Hh, W))
    pe[dm + 1::2] = np.broadcast_to(np.cos(ph * div).T[:, :, None], (dm // 2, Hh, W))
    return pe.transpose(1, 2, 0)  # (H, W, D)


def _wrap_idx(vec):
    """indirect_copy index layout: index j lives at (partition j%16,
    slot j//16), replicated for each 16-partition group."""
    w = np.zeros((128, L // 16), np.uint16)
    blkv = vec.reshape(L // 16, 16).T.astype(np.uint16)
    for g in range(128 // 16):
        w[g * 16:(g + 1) * 16, :] = blkv
    return w


def _bf(x):
    return np.ascontiguousarray(x).astype(ml_dtypes.bfloat16)


def make_in_maps(inputs):
    x = np.asarray(inputs["x"], np.float32)
    y = np.asarray(inputs["y"]).astype(np.int64)
    ci = np.asarray(inputs["cell_indices"]).astype(np.int64)
    cellEB = np.asarray(inputs["cellEB"], np.float32)
    CpGEB = np.asarray(inputs["CpGEB"], np.float32)
    fcc_w = np.asarray(inputs["fcc_w"], np.float32)
    fcc_b = np.asarray(inputs["fcc_b"], np.float32)
    ln_g = np.asarray(inputs["ln_g"], np.float32)
    ln_b = np.asarray(inputs["ln_b"], np.float32)
    in_proj_w = np.asarray(inputs["in_proj_w"], np.float32)
    conv_w = np.asarray(inputs["conv_w"], np.float32)
    conv_b = np.asarray(inputs["conv_b"], np.float32)
    x_proj_w = np.asarray(inputs["x_proj_w"], np.float32)
    dt_proj_w = np.asarray(inputs["dt_proj_w"], np.float32)
    dt_proj_b = np.asarray(inputs["dt_proj_b"], np.float32)
    A_log = np.asarray(inputs["A_log"], np.float32)
    D_param = np.asarray(inputs["D_param"], np.float32)
    out_proj_w = np.asarray(inputs["out_proj_w"], np.float32)

    pos = _pos_enc(3 * DIM, NSITE, NCELL)          # (site, cell, 96)
    pos_t1 = pos.reshape(L, 3 * DIM)

    # embedding gather + concat (site-major t1 ordering)
    emb = np.concatenate([
        CpGEB[y],                                   # (B, site, cell, 32)
        np.broadcast_to(cellEB[ci][:, None], (B, NSITE, NCELL, DIM)),
        np.broadcast_to(x[:, :, None, :], (B, NSITE, NCELL, DIM)),
    ], axis=-1).reshape(B, L, 3 * DIM)

    Apos = np.exp(A_log)                            # |A| = -A, (DI, DS)

    idx_id = np.arange(L, dtype=np.int64)
    idx_rv = idx_id[::-1].copy()
    # t2 permutation: u2[v] = h2_canon[perm0[v]], v = c*NSITE + s
    v = np.arange(L)
    c_, s_ = v // NSITE, v % NSITE
    perm0 = s_ * NCELL + c_

    pvec = np.arange(128)

    in_maps = []
    for core in range(N_CORES):
        b = core >> 2
        dirb = (core >> 1) & 1
        dh = core & 1
        e = emb[b] if dirb == 0 else emb[b][::-1]
        p1 = pos_t1 if dirb == 0 else pos_t1[::-1]
        # channel permutation: our 64 channels first
        ordc = np.r_[dh * DH:(dh + 1) * DH, (1 - dh) * DH:(2 - dh) * DH]
        ours = ordc[:DH]

        # x_proj stationaries with 4x row replication over state-32-blocks
        xb = np.zeros((DI, NQ * 128), np.float32)
        xct = np.zeros((DI, NQ * 128), np.float32)
        for q in range(NQ):
            rowsB = -x_proj_w[DR + q * 32:DR + (q + 1) * 32][:, ordc].T  # (DI,32)
            rowsC = x_proj_w[DR + DS + q * 32:DR + DS + (q + 1) * 32][:, ordc].T
            xb[:, q * 128:(q + 1) * 128] = np.tile(rowsB, (1, 4))
            xct[:, q * 128:(q + 1) * 128] = np.tile(rowsC, (1, 4))

        # |A| scale table: partition p of group g, eighth q ->
        # channel ours[g*4 + p//32], state q*32 + p%32
        apos = np.zeros((128, NG * NQ), np.float32)
        for g in range(NG):
            ch = ours[g * 4 + pvec // 32]
            for q in range(NQ):
                st = q * 32 + pvec % 32
                apos[:, g * NQ + q] = Apos[ch, st]

        m = {
            "emb_T": np.ascontiguousarray(e.T),
            "pos_T": np.ascontiguousarray(p1.T),
            "fccT": _bf(fcc_w.T),
            "fccb": fcc_b.reshape(DM, 1),
            "inpxT": _bf(in_proj_w[ordc, :].T),
            "inpzT": _bf(in_proj_w[DI + ours, :].T),
            "convW": np.ascontiguousarray(conv_w[ordc, 0, :]),
            "convB": conv_b[ordc].reshape(DI, 1),
            "xprojD": _bf(x_proj_w[0:DR][:, ordc].T),
            "xprojB": _bf(xb),
            "xprojC": _bf(xct),
            "dtwT": _bf(dt_proj_w[ours, :].T),
            "ndtb": (-dt_proj_b[ours]).reshape(DH, 1),
            "aposT": apos,
            "bc4": _bf((pvec[None, :] // 32) == np.arange(4)[:, None]),
            "dcol": D_param[ours].reshape(DH, 1),
            "lng": ln_g.reshape(DM, 1),
            "lnb": ln_b.reshape(DM, 1),
            "outprojT": _bf(out_proj_w[:, ours].T * 0.5),
            "idx_rev": _wrap_idx(idx_id if dirb == 0 else idx_rv),
            "idx2": _wrap_idx(perm0 if dirb == 0 else perm0[::-1]),
        }
        in_maps.append(m)
    return in_maps


def postprocess(results):
    out = np.zeros((B, NSITE, NCELL, DM), np.float32)
    for b, core in ((0, 0), (1, 4)):
        h3 = results[core]["out"]                   # (DM, L) t2-canonical
        seq = h3.T.reshape(NCELL, NSITE, DM)        # v = c*NSITE + s
        out[b] = seq.transpose(1, 0, 2)
    return out



# ---------------------------------------------------------------------------
# cached PJRT runner (built once per process; repeat kernel() calls are fast)
# ---------------------------------------------------------------------------
import time

import jax
from jax.sharding import Mesh, PartitionSpec
from jax.experimental.shard_map import shard_map

from concourse.bass2jax import _bass_exec_p, install_neuronx_cc_hook, partition_id_tensor


class Runner:
    def __init__(self, nc, in_maps, n_cores=8):
        install_neuronx_cc_hook()
        self.n_cores = n_cores
        partition_name = nc.partition_id_tensor.name if nc.partition_id_tensor else None
        in_names, out_names, out_avals, zero_outs = [], [], [], []
        for alloc in nc.m.functions[0].allocations:
            if not isinstance(alloc, mybir.MemoryLocationSet):
                continue
            name = alloc.memorylocations[0].name
            if alloc.kind == "ExternalInput":
                if name != partition_name:
                    in_names.append(name)
            elif alloc.kind == "ExternalOutput":
                out_names.append(name)
                shape = tuple(alloc.tensor_shape)
                dtype = mybir.dt.np(alloc.dtype)
                out_avals.append(jax.core.ShapedArray(shape, dtype))
                zero_outs.append(np.zeros(shape, dtype))
        n_params = len(in_names)
        n_outs = len(out_avals)
        all_in_names = list(in_names) + out_names
        if partition_name is not None:
            all_in_names.append(partition_name)
        donate = tuple(range(n_params, n_params + n_outs))

        def _body(*args):
            operands = list(args)
            if partition_name is not None:
                operands.append(partition_id_tensor())
            outs = _bass_exec_p.bind(
                *operands,
                out_avals=tuple(out_avals),
                in_names=tuple(all_in_names),
                out_names=tuple(out_names),
                lowering_input_output_aliases=(),
                sim_require_finite=True,
                sim_require_nnan=True,
                nc=nc,
            )
            return tuple(outs)

        devices = jax.devices()[:n_cores]
        mesh = Mesh(np.asarray(devices), ("core",))
        in_specs = (PartitionSpec("core"),) * (n_params + n_outs)
        out_specs = (PartitionSpec("core"),) * n_outs
        self.f = jax.jit(
            shard_map(_body, mesh=mesh, in_specs=in_specs,
                      out_specs=out_specs, check_rep=False),
            donate_argnums=donate, keep_unused=True)
        self.in_names = in_names
        self.n_params = n_params
        self.sharding = jax.sharding.NamedSharding(mesh, PartitionSpec("core"))
        self.set_inputs(in_maps)
        zshapes = [(n_cores * z.shape[0], *z.shape[1:]) for z in zero_outs]
        zdt = [z.dtype for z in zero_outs]

        def _mkzeros():
            return tuple(jax.numpy.zeros(s, d) for s, d in zip(zshapes, zdt))

        self.mkzeros = jax.jit(_mkzeros, out_shardings=(self.sharding,) * n_outs)
        self.out_names = out_names
        self.out_avals = out_avals

    def set_inputs(self, in_maps):
        per_core = [[np.asarray(m[n]) for n in self.in_names] for m in in_maps]
        concat_in = [
            np.concatenate([per_core[c][i] for c in range(self.n_cores)], axis=0)
            for i in range(self.n_params)
        ]
        self.inputs_dev = [jax.device_put(a, self.sharding) for a in concat_in]

    def run(self):
        z = self.mkzeros()
        jax.block_until_ready(z)
        t0 = time.time()
        outs = self.f(*self.inputs_dev, *z)
        jax.block_until_ready(outs)
        dt = time.time() - t0
        return outs, dt

    def results(self, outs):
        res = []
        for c in range(self.n_cores):
            m = {}
            for i, name in enumerate(self.out_names):
                a = np.asarray(outs[i])
                m[name] = a.reshape(self.n_cores, *self.out_avals[i].shape)[c]
            res.append(m)
        return res

    def bench(self, warmup=2, iters=12):
        for _ in range(warmup):
            self.run()
        ts = []
        for _ in range(iters):
            _, dt = self.run()
            ts.append(dt)
        ts.sort()
        return ts[len(ts) // 2], ts[0]


_cache = {}


def _get_nc(nrep=1):
    if nrep not in _cache:
        _cache[nrep] = build_bass(nrep)
    return _cache[nrep]


_runner_cache = {}


def get_runner(inputs, nrep=1):
    key = nrep
    if key not in _runner_cache:
        _runner_cache[key] = Runner(_get_nc(nrep), make_in_maps(inputs), N_CORES)
    return _runner_cache[key]


def kernel(**inputs) -> np.ndarray:
    r = get_runner(inputs, 1)
    # refresh device inputs in case the caller passes different data
    in_maps = make_in_maps(inputs)
    r.set_inputs(in_maps)
    outs, _ = r.run()
    return postprocess(r.results(outs))
